# revision 1
# baseline (speedup 1.0000x reference)
"""Trainium2 Bass kernel for AttentionSimple (linear/kernelized attention).

Computes, for x:[B,N,C], w_qkv:[C,3C], w_proj:[C,C], b_proj:[C]:
    qkv = x @ w_qkv -> split q,k,v per head (H=12, D=64)
    kv  = (k^T v) * D^-0.5          per (b, h)     [D, D]
    out = gelu(q) @ gelu(kv)        per (b, h)     [N, D]
    y   = out @ w_proj + b_proj

Sharding: data-parallel over batch B=16 across 8 NeuronCores (2 batches/core).
All matmuls run in bf16 with fp32 PSUM accumulation.

Dataflow per core (per batch b):
  pass 1: x tiles -> bf16 -> x^T via PE transposes (batched strided DVE
          evacuation); k,v in natural [token, d] layout (lhsT = x^T chunk);
          per-head-pair k^T v accumulated into a persistent PSUM tile; q^T
          computed directly transposed (lhsT = w_q chunk, rhs = x^T) with
          gelu fused into the PSUM evacuation; gelu(kv * scale) written into
          block-diagonal [128,128] pair tiles.
  pass 2: attention o^T per pair as one matmul with the block-diagonal
          gelu(kv) stationary; projection consumes o^T directly
          (natural-layout output); bias added on gpsimd; contiguous DMA out.

Self-contained: hardcodes shapes; builds the Bass program, runs it SPMD on
cores 0-7 via bass_utils.run_bass_kernel_spmd, returns the gathered output.
"""

import numpy as np

import concourse.bacc as bacc
import concourse.bass as bass
import concourse.mybir as mybir
import concourse.tile as tile
from concourse import masks
from concourse.bass_utils import run_bass_kernel_spmd

F32 = mybir.dt.float32
BF16 = mybir.dt.bfloat16
GELU = mybir.ActivationFunctionType.Gelu
COPY = mybir.ActivationFunctionType.Copy
PSUM = bass.MemorySpace.PSUM

B, N, C = 16, 4096, 768
H, D = 12, 64
SCALE = D**-0.5
NCORES = 8
BPC = B // NCORES  # batches per core
CCH = C // 128  # 6 column chunks of 128
NTS = N // 512  # 8 slices of 512 tokens
NPAIR = H // 2  # 6 head pairs (128 cols each)


def _build_program():
    nc = bacc.Bacc("TRN2", target_bir_lowering=False, debug=False)

    x_d = nc.dram_tensor("x", [BPC, N, C], F32, kind="ExternalInput").ap()
    wq_d = nc.dram_tensor("w_qkv", [C, 3 * C], F32, kind="ExternalInput").ap()
    wp_d = nc.dram_tensor("w_proj", [C, C], F32, kind="ExternalInput").ap()
    bp_d = nc.dram_tensor("b_proj", [C], F32, kind="ExternalInput").ap()
    y_d = nc.dram_tensor("y", [BPC, N, C], F32, kind="ExternalOutput").ap()

    with tile.TileContext(nc) as tc:
        with (
            tc.tile_pool(name="weights", bufs=1) as wpool,
            tc.tile_pool(name="acts", bufs=1) as apool,
            tc.tile_pool(name="xin", bufs=8) as xpool,
            tc.tile_pool(name="xt", bufs=4) as xtpool,
            tc.tile_pool(name="ot", bufs=4) as otpool,
            tc.tile_pool(name="yout", bufs=3) as ypool,
            tc.tile_pool(name="ps_mm", bufs=3, space=PSUM) as ps_mm,
        ):
            # ---- constants first (cheap, unblock transposes) ----
            ident = wpool.tile([128, 128], BF16)
            masks.make_identity(nc, ident[:])

            # ---- prefetch the first token slice before the big weight DMAs
            # so the SWDGE rings deliver x(0,0) immediately and the PE can
            # start transposing while weights stream in.
            def load_x(b, ts):
                tiles = []
                for tc4 in range(4):
                    t0 = ts * 512 + tc4 * 128
                    x_bf = xpool.tile([128, C], BF16, tag="x_bf")
                    nc.gpsimd.dma_start(x_bf[:], x_d[b, t0 : t0 + 128, :])
                    tiles.append(x_bf)
                return tiles

            x_pre = load_x(0, 0)

            # ---- weights: k/v 512-col slices first (needed first), then q,
            # proj ----
            w_qkv = wpool.tile([128, CCH, 3 * C], BF16)  # 27.6KB/part
            w_proj = wpool.tile([128, CCH, C], BF16)  # 9.2KB/part
            for lo, hi in ((0, 512), (512, 768)):  # q slices (needed first)
                for cch in range(CCH):
                    nc.gpsimd.dma_start(
                        w_qkv[:, cch, lo:hi], wq_d[cch * 128 : (cch + 1) * 128, lo:hi]
                    )
            # Remaining weight loads are deferred into the first batch's
            # ts-loop (3 per slice) so their issue cost and ring bandwidth
            # don't delay the x-tile prefetches. Deps still order correctly.
            b_bc = wpool.tile([128, C], F32)
            deferred_w = []
            for cch in range(CCH):  # v part (A-stage), then k part (kv-stage)
                deferred_w.append(
                    (w_qkv[:, cch, 2 * C :], wq_d[cch * 128 : (cch + 1) * 128, 2 * C :])
                )
            for cch in range(CCH):
                deferred_w.append(
                    (
                        w_qkv[:, cch, C : 2 * C],
                        wq_d[cch * 128 : (cch + 1) * 128, C : 2 * C],
                    )
                )
            for cch in range(CCH):
                deferred_w.append(
                    (w_proj[:, cch, :], wp_d[cch * 128 : (cch + 1) * 128, :])
                )
            deferred_w.append((b_bc[:], bp_d.unsqueeze(0).partition_broadcast(128)))
            deferred_w.reverse()
            for b in range(BPC):
                # gqT: q^T with gelu applied, [c=768, t=4096] as 6 chunks
                gqT = apool.tile([128, CCH, N], BF16, tag="gqT")
                # gkv: per-pair block-diagonal [128,128] with gelu(kv*scale)
                # of the even head at [0:64,0:64] and odd head at [64:,64:]
                gkv = apool.tile([128, NPAIR, 128], BF16, tag="gkv")
                # Gram matrix G = x^T x (bf16), used for kv = W_k^T G W_v
                G_sb = apool.tile([128, CCH, C], BF16, tag="G")

                # ===== pass 1a: q^T + upper-triangular G rows 0..2 ==========
                with tc.tile_pool(name="ps_gA", bufs=1, space=PSUM) as ps_gA:
                    g_acc = [
                        ps_gA.tile([128, C - ci * 128], F32, tag=f"g{ci}", name=f"gA{ci}")
                        for ci in range(3)
                    ]
                    for ts in range(NTS):
                        xT = xtpool.tile([128, CCH, 512], BF16)
                        x_tiles = x_pre
                        if ts + 1 < NTS:
                            x_pre = load_x(b, ts + 1)
                        elif b + 1 < BPC:
                            x_pre = load_x(b + 1, 0)
                        for _ in range(3):
                            if deferred_w:
                                dst, srcap = deferred_w.pop()
                                nc.gpsimd.dma_start(dst, srcap)
                        if ts == NTS - 1:
                            xhf_pre = []
                            for tc4 in range(4):
                                t0 = tc4 * 128
                                x_hf = xpool.tile(
                                    [128, C - 384], BF16, tag="x_hf"
                                )
                                nc.gpsimd.dma_start(
                                    x_hf[:], x_d[b, t0 : t0 + 128, 384:C]
                                )
                                xhf_pre.append(x_hf)
                        for tc4 in range(4):
                            x_bf = x_tiles[tc4]
                            tr = ps_mm.tile([128, CCH * 128], BF16, tag="pmm")
                            for cch in range(CCH):
                                nc.tensor.transpose(
                                    tr[:, cch * 128 : (cch + 1) * 128],
                                    x_bf[:, cch * 128 : (cch + 1) * 128],
                                    ident[:],
                                )
                            nc.vector.tensor_copy(
                                xT[:, :, tc4 * 128 : tc4 * 128 + 128],
                                tr[:].rearrange("p (c f) -> p c f", c=CCH),
                            )
                            # G rows ci, cols [ci*128:768), accumulated over
                            # all 32 token chunks; each 512/256-col split owns
                            # its psum bank so start=(first chunk) is safe.
                            first = ts == 0 and tc4 == 0
                            last = ts == NTS - 1 and tc4 == 3
                            for ci in range(3):
                                w = C - ci * 128
                                for lo in range(0, w, 512):
                                    hi = min(lo + 512, w)
                                    nc.tensor.matmul(
                                        g_acc[ci][:, lo:hi],
                                        x_bf[:, ci * 128 : (ci + 1) * 128],
                                        x_bf[:, ci * 128 + lo : ci * 128 + hi],
                                        start=first,
                                        stop=last,
                                        skip_group_check=True,
                                    )
                        # ---- q^T chunks with fused gelu (last ts deferred
                        # into pass 1b to fill its DMA-paced PE gaps) ----
                        if ts == NTS - 1:
                            xT_last = xT
                            continue
                        for jch in range(CCH):
                            pq = ps_mm.tile([128, 512], F32, tag="pmm")
                            for cch in range(CCH):
                                nc.tensor.matmul(
                                    pq[:],
                                    w_qkv[:, cch, jch * 128 : (jch + 1) * 128],
                                    xT[:, cch, 0:512],
                                    start=(cch == 0),
                                    stop=(cch == CCH - 1),
                                )
                            nc.scalar.activation(
                                gqT[:, jch, ts * 512 : ts * 512 + 512], pq[:], GELU
                            )
                    for ci in range(3):
                        nc.vector.tensor_copy(
                            G_sb[:, ci, ci * 128 : C], g_acc[ci][:]
                        )

                # ===== pass 1b: G rows 3..5 (x cols 384: reloaded) ==========
                early_mirrors = [
                    (1, 0), (2, 0), (2, 1), (3, 0), (3, 1), (3, 2),
                    (4, 0), (4, 1), (4, 2), (5, 0), (5, 1), (5, 2),
                ][::-1]
                with tc.tile_pool(name="ps_gB", bufs=1, space=PSUM) as ps_gB:
                    g_accB = [
                        ps_gB.tile([128, C - ci * 128], F32, tag=f"g{ci}", name=f"gB{ci}")
                        for ci in range(3, CCH)
                    ]
                    def load_xhf(ts):
                        tiles = []
                        for tc4 in range(4):
                            t0 = ts * 512 + tc4 * 128
                            x_hf = xpool.tile([128, C - 384], BF16, tag="x_hf")
                            nc.gpsimd.dma_start(
                                x_hf[:], x_d[b, t0 : t0 + 128, 384:C]
                            )
                            tiles.append(x_hf)
                        return tiles

                    def deferred_q(jch):
                        pq = ps_mm.tile([128, 512], F32, tag="pmm", name="pqd")
                        for cch in range(CCH):
                            nc.tensor.matmul(
                                pq[:],
                                w_qkv[:, cch, jch * 128 : (jch + 1) * 128],
                                xT_last[:, cch, 0:512],
                                start=(cch == 0),
                                stop=(cch == CCH - 1),
                            )
                        nc.scalar.activation(
                            gqT[:, jch, (NTS - 1) * 512 : NTS * 512], pq[:], GELU
                        )

                    for ts in range(NTS):
                        xhf_tiles = xhf_pre
                        if ts + 1 < NTS:
                            xhf_pre = load_xhf(ts + 1)
                        if ts < CCH:
                            deferred_q(ts)
                        # fill the DMA-paced P1b with mirror transposes whose
                        # sources (G rows 0-2) were finished in pass 1a
                        for _ in range(2):
                            if early_mirrors:
                                i, j = early_mirrors.pop()
                                pt = ps_mm.tile(
                                    [128, 128], BF16, tag="pmm", name=f"pt{i}{j}"
                                )
                                nc.tensor.transpose(
                                    pt[:],
                                    G_sb[:, j, i * 128 : i * 128 + 128],
                                    ident[:],
                                )
                                nc.vector.tensor_copy(
                                    G_sb[:, i, j * 128 : j * 128 + 128], pt[:]
                                )
                        for tc4 in range(4):
                            x_hf = xhf_tiles[tc4]
                            first = ts == 0 and tc4 == 0
                            last = ts == NTS - 1 and tc4 == 3
                            for k, ci in enumerate(range(3, CCH)):
                                off = ci * 128 - 384
                                nc.tensor.matmul(
                                    g_accB[k][:],
                                    x_hf[:, off : off + 128],
                                    x_hf[:, off:],
                                    start=first,
                                    stop=last,
                                    skip_group_check=True,
                                )
                    # parallelize the tail evacs across DVE and ACT so the
                    # A-stage psum banks (reused from here) free up sooner
                    nc.vector.tensor_copy(G_sb[:, 3, 384:C], g_accB[0][:])
                    nc.scalar.activation(G_sb[:, 4, 512:C], g_accB[1][:], COPY)
                    nc.vector.tensor_copy(G_sb[:, 5, 640:C], g_accB[2][:])
                for n, (i, j) in enumerate(((4, 3), (5, 3), (5, 4))):
                    pt = ps_mm.tile([128, 128], BF16, tag="pmm", name=f"pt{i}{j}")
                    nc.tensor.transpose(
                        pt[:], G_sb[:, j, i * 128 : i * 128 + 128], ident[:]
                    )
                    if n % 2 == 0:
                        nc.vector.tensor_copy(
                            G_sb[:, i, j * 128 : j * 128 + 128], pt[:]
                        )
                    else:
                        nc.scalar.activation(
                            G_sb[:, i, j * 128 : j * 128 + 128], pt[:], COPY
                        )


                # ---- A = G @ W_v  (contraction over c) ----
                A_sb = apool.tile([128, CCH, C], BF16, tag="A")
                with tc.tile_pool(name="ps_A", bufs=2, space=PSUM) as ps_A:
                    for cp in range(CCH):
                        pA = ps_A.tile([128, C], F32, tag="pA")
                        for lo, hi in ((0, 512), (512, 768)):
                            for cch in range(CCH):
                                nc.tensor.matmul(
                                    pA[:, lo:hi],
                                    G_sb[:, cch, cp * 128 : (cp + 1) * 128],
                                    w_qkv[:, cch, 2 * C + lo : 2 * C + hi],
                                    start=(cch == 0),
                                    stop=(cch == CCH - 1),
                                    skip_group_check=True,
                                )
                        nc.vector.tensor_copy(A_sb[:, cp, :], pA[:])

                # ---- kv pairs = W_k_pair^T @ A_pair, then gelu(kv*scale) ----
                with tc.tile_pool(name="ps_kv", bufs=1, space=PSUM) as ps_kv:
                    kv_acc = ps_kv.tile([128, NPAIR * 128], F32)
                    for pr in range(NPAIR):
                        psl = slice(pr * 128, pr * 128 + 128)
                        for cch in range(CCH):
                            # start=True clears has_written for the WHOLE psum
                            # bank: only the first matmul touching each bank
                            # sets it (bank0: pairs 0-3, bank1: pairs 4-5).
                            nc.tensor.matmul(
                                kv_acc[:, psl],
                                w_qkv[:, cch, C + pr * 128 : C + (pr + 1) * 128],
                                A_sb[:, cch, pr * 128 : (pr + 1) * 128],
                                start=(cch == 0 and pr in (0, 4)),
                                stop=(cch == CCH - 1),
                                skip_group_check=True,
                            )
                    # ---- gelu(kv * scale) into block-diagonal pair tiles ----
                    nc.gpsimd.memset(gkv[:], 0.0)
                    for pr in range(NPAIR):
                        c0 = pr * 128
                        nc.scalar.activation(
                            gkv[0:64, pr, 0:64],
                            kv_acc[0:64, c0 : c0 + 64],
                            GELU,
                            scale=SCALE,
                        )
                        nc.scalar.activation(
                            gkv[64:128, pr, 64:128],
                            kv_acc[64:128, c0 + 64 : c0 + 128],
                            GELU,
                            scale=SCALE,
                        )

                # ================= pass 2: attention + projection ===========
                with tc.tile_pool(name="ps_p2", bufs=3, space=PSUM) as ps_p2:
                  for ts in range(NTS):
                    tq = slice(ts * 512, ts * 512 + 512)
                    oT = otpool.tile([128, NPAIR, 512], BF16)
                    for pr in range(NPAIR):
                        po = ps_p2.tile([128, 512], F32, tag="po")
                        nc.tensor.matmul(po[:], gkv[:, pr, :], gqT[:, pr, tq])
                        if pr % 2 == 0:
                            nc.vector.tensor_copy(oT[:, pr, :], po[:])
                        else:
                            nc.scalar.activation(oT[:, pr, :], po[:], COPY)
                    for tc4 in range(4):
                        tsl = slice(tc4 * 128, tc4 * 128 + 128)
                        py0 = ps_mm.tile([128, 512], F32, tag="pmm")
                        py1 = ps_mm.tile([128, 256], F32, tag="pmm")
                        for cch in range(CCH):
                            last = cch == CCH - 1
                            nc.tensor.matmul(
                                py0[:],
                                oT[:, cch, tsl],
                                w_proj[:, cch, 0:512],
                                start=(cch == 0),
                                stop=last,
                                skip_group_check=True,
                            )
                            nc.tensor.matmul(
                                py1[:],
                                oT[:, cch, tsl],
                                w_proj[:, cch, 512:768],
                                start=(cch == 0),
                                stop=last,
                                skip_group_check=True,
                            )
                        y_sb = ypool.tile([128, C], F32)
                        nc.vector.tensor_add(y_sb[:, 0:512], py0[:], b_bc[:, 0:512])
                        nc.scalar.activation(y_sb[:, 512:768], py1[:], COPY)
                        nc.gpsimd.tensor_add(
                            y_sb[:, 512:768], y_sb[:, 512:768], b_bc[:, 512:768]
                        )
                        t0 = ts * 512 + tc4 * 128
                        nc.sync.dma_start(y_d[b, t0 : t0 + 128, :], y_sb[:])

    nc.compile()
    return nc


_cached_nc = None


def kernel(x, w_qkv, w_proj, b_proj):
    global _cached_nc
    if _cached_nc is None:
        _cached_nc = _build_program()
    nc = _cached_nc

    x = np.ascontiguousarray(x, dtype=np.float32)
    in_maps = [
        {
            "x": x[i * BPC : (i + 1) * BPC],
            "w_qkv": np.asarray(w_qkv, dtype=np.float32),
            "w_proj": np.asarray(w_proj, dtype=np.float32),
            "b_proj": np.asarray(b_proj, dtype=np.float32),
        }
        for i in range(NCORES)
    ]
    last_err = None
    for _attempt in range(3):
        try:
            res = run_bass_kernel_spmd(nc, in_maps, core_ids=list(range(NCORES)))
            out = np.concatenate(
                [res.results[i]["y"] for i in range(NCORES)], axis=0
            )
            return out.astype(np.float32)
        except Exception as e:  # transient NRT device errors recover on retry
            last_err = e
    raise last_err



# revision 13
# speedup vs baseline: 1.1212x; 1.1212x over previous
"""Trainium2 Bass kernel for AttentionSimple (linear/kernelized attention).

Computes, for x:[B,N,C], w_qkv:[C,3C], w_proj:[C,C], b_proj:[C]:
    qkv = x @ w_qkv -> split q,k,v per head (H=12, D=64)
    kv  = (k^T v) * D^-0.5          per (b, h)     [D, D]
    out = gelu(q) @ gelu(kv)        per (b, h)     [N, D]
    y   = out @ w_proj + b_proj

Sharding: data-parallel over batch B=16 across 8 NeuronCores (2 batches/core).
All matmuls run in bf16 with fp32 PSUM accumulation.

Algorithm per core (per batch b), using the Gram trick
kv^T = W_v^T (x^T x) W_k (G = x^T x symmetric) and folding the attention
into the projection: y = gelu(q) @ W' with W'_h = gelu(kv)_h @ w_proj_h:

  pass 1a (per 512-token slice): x loaded once as lo[0:384]/hi[384:768]
      bf16 tiles (hi retained for pass 1b); x^T via PE transposes; G rows
      0-2 (upper triangle) accumulated in one packed 4-bank PSUM region;
      q^T chunks (lhsT = W_q chunk, rhs = x^T) with gelu fused into the
      ACT evacuation.
  pass 1b: G rows 3-5 from the retained hi tiles (no re-DMA); G evacs and
      the 15 mirror transposes interleaved.
  chain:  A = G @ W_k; kv^T pairs = W_v^T A; gelu(kv^T * scale) into
      block-diagonal pair tiles; W'_pair = gkvT^T @ w_proj rows.
  pass 2: y[tokens, C] = sum_pr gqT_pr^T @ W'_pair + bias; contiguous DMA.

Self-contained: hardcodes shapes; builds the Bass program, runs it SPMD on
cores 0-7 via bass_utils.run_bass_kernel_spmd, returns the gathered output.
"""

import numpy as np

import concourse.bacc as bacc
import concourse.bass as bass
import concourse.mybir as mybir
import concourse.tile as tile
from concourse import masks
from concourse.bass_utils import run_bass_kernel_spmd

F32 = mybir.dt.float32
BF16 = mybir.dt.bfloat16
GELU = mybir.ActivationFunctionType.Gelu
COPY = mybir.ActivationFunctionType.Copy
PSUM = bass.MemorySpace.PSUM

B, N, C = 16, 4096, 768
H, D = 12, 64
SCALE = D**-0.5
NCORES = 8
BPC = B // NCORES  # batches per core
CCH = C // 128  # 6 column chunks of 128
NTS = N // 512  # 8 slices of 512 tokens
NPAIR = H // 2  # 6 head pairs (128 cols each)
XLO = 384  # x cols [0:384) in recycled lo tiles, [384:768) retained hi tiles

# G rows 0-2 packed into one [128, 1920] PSUM region (banks of 512 f32).
# (row, psum_lo, src_lo, width); no MM crosses a 512-col PSUM bank or the
# lo/hi tile boundary at src col 384.
G_A_SPLITS = [
    (0, 0, 0, 384),
    (0, 384, 384, 128),
    (0, 512, 512, 256),
    (1, 768, 128, 256),
    (1, 1024, 384, 384),
    (2, 1408, 256, 128),
    (2, 1536, 384, 384),
]
# G rows 3-5: tile1 holds row3 @[0:384) + row5 @[384:512), tile2 row4 @[0:256)
G_B_SPLITS = [
    (3, 0, 0, 0, 384),  # (row, tile_idx, psum_lo, src_lo(in hi), width)
    (4, 1, 0, 512 - XLO, 256),
    (5, 0, 384, 640 - XLO, 128),
]
MIRRORS_EARLY = [(i, j) for i in range(1, CCH) for j in range(min(i, 3))]
MIRRORS_LATE = [(4, 3), (5, 3), (5, 4)]


DEBUG_DUMPS = False


def _build_program():
    nc = bacc.Bacc("TRN2", target_bir_lowering=False, debug=False)

    dbg = {}
    if DEBUG_DUMPS:
        dbg["G"] = nc.dram_tensor("G_dbg", [128, CCH, C], BF16, kind="ExternalOutput").ap()
        dbg["gq"] = nc.dram_tensor("gq_dbg", [128, CCH, 512], BF16, kind="ExternalOutput").ap()
        dbg["A"] = nc.dram_tensor("A_dbg", [128, CCH, C], BF16, kind="ExternalOutput").ap()
        dbg["W"] = nc.dram_tensor("W_dbg", [128, NPAIR, C], BF16, kind="ExternalOutput").ap()

    x_d = nc.dram_tensor("x", [BPC, N, C], F32, kind="ExternalInput").ap()
    wq_d = nc.dram_tensor("w_qkv", [C, 3 * C], F32, kind="ExternalInput").ap()
    wp_d = nc.dram_tensor("w_proj", [C, C], F32, kind="ExternalInput").ap()
    bp_d = nc.dram_tensor("b_proj", [C], F32, kind="ExternalInput").ap()
    y_d = nc.dram_tensor("y", [BPC, N, C], F32, kind="ExternalOutput").ap()

    with tile.TileContext(nc) as tc:
        with (
            tc.tile_pool(name="weights", bufs=1) as wpool,
            tc.tile_pool(name="acts", bufs=1) as apool,
            tc.tile_pool(name="gq", bufs=8) as gqpool,
            tc.tile_pool(name="xlo", bufs=8) as xpool,
            tc.tile_pool(name="xhi", bufs=36) as xhipool,
            tc.tile_pool(name="xt", bufs=3) as xtpool,
            tc.tile_pool(name="yout", bufs=3) as ypool,
            tc.tile_pool(name="ps_tr", bufs=2, space=PSUM) as ps_tr,
            tc.tile_pool(name="ps_pq", bufs=2, space=PSUM) as ps_pq,
        ):
            # ---- identity for PE transposes ----
            ident = wpool.tile([128, 128], BF16)
            masks.make_identity(nc, ident[:])

            # ---- HAM warmup: dense dummy matmuls so the PE clock-gate
            # flips to 8/8 ~3.6us in instead of ~15us.
            warm = ps_pq.tile([128, 512], F32, tag="pq", name="warm")
            for _ in range(28):
                nc.tensor.matmul(warm[:, 0:128], ident[:], ident[:], start=True,
                                 stop=True, skip_group_check=True)

            # ---- x prefetch helpers (lo recycled, hi retained per batch) --
            def load_x(b, ts):
                tiles = []
                for tc4 in range(4):
                    t0 = ts * 512 + tc4 * 128
                    x_lo = xpool.tile([128, XLO], BF16, tag="x_lo")
                    nc.gpsimd.dma_start(x_lo[:], x_d[b, t0 : t0 + 128, 0:XLO])
                    x_hi = xhipool.tile([128, C - XLO], BF16, tag="x_hi")
                    nc.gpsimd.dma_start(x_hi[:], x_d[b, t0 : t0 + 128, XLO:C])
                    tiles.append((x_lo, x_hi))
                return tiles

            x_pre = load_x(0, 0)

            # ---- weights: q slices first (needed first); k, v, proj and
            # bias deferred into the pass-1a ts loop.
            w_qkv = wpool.tile([128, CCH, 3 * C], BF16)
            w_proj = wpool.tile([128, CCH, C], BF16)
            for lo, hi in ((0, 512), (512, 768)):
                for cch in range(CCH):
                    nc.gpsimd.dma_start(
                        w_qkv[:, cch, lo:hi], wq_d[cch * 128 : (cch + 1) * 128, lo:hi]
                    )
            b_bc = wpool.tile([128, C], F32)
            deferred_w = []
            for cch in range(CCH):  # k part (A-stage consumes first)
                deferred_w.append(
                    (w_qkv[:, cch, C : 2 * C], wq_d[cch * 128 : (cch + 1) * 128, C : 2 * C])
                )
            for cch in range(CCH):  # v part (kv-stage)
                deferred_w.append(
                    (w_qkv[:, cch, 2 * C :], wq_d[cch * 128 : (cch + 1) * 128, 2 * C :])
                )
            for cch in range(CCH):
                deferred_w.append(
                    (w_proj[:, cch, :], wp_d[cch * 128 : (cch + 1) * 128, :])
                )
            deferred_w.append((b_bc[:], bp_d.unsqueeze(0).partition_broadcast(128)))
            deferred_w.reverse()

            gkvT = apool.tile([128, NPAIR, 128], BF16, tag="gkv")

            for b in range(BPC):
                # gqT: gelu(q)^T, [c=768, t=4096] as 6 chunks, per-ts tiles
                gq_ts = []
                # G (bf16, both triangles after mirrors)
                G_sb = apool.tile([128, CCH, C], BF16, tag="G")
                x_hi_keep = []  # [32][tile] retained hi tiles for pass 1b

                # ===== pass 1a: x^T, G rows 0-2, gelu(q)^T ==================
                with tc.tile_pool(name="ps_gA", bufs=1, space=PSUM) as ps_gA:
                    g_acc = ps_gA.tile([128, 1920], F32, name="gA")
                    for ts in range(NTS):
                        xT = xtpool.tile([128, CCH, 512], BF16)
                        x_tiles = x_pre
                        if ts + 1 < NTS:
                            x_pre = load_x(b, ts + 1)
                        elif b + 1 < BPC:
                            x_pre = load_x(b + 1, 0)
                        for _ in range(3):
                            if deferred_w:
                                dst, srcap = deferred_w.pop()
                                nc.gpsimd.dma_start(dst, srcap)
                        for tc4 in range(4):
                            x_lo, x_hi = x_tiles[tc4]
                            x_hi_keep.append(x_hi)
                            tr = ps_tr.tile([128, CCH * 128], BF16, tag="tr")
                            for cch in range(CCH):
                                src = (
                                    x_lo[:, cch * 128 : (cch + 1) * 128]
                                    if cch < 3
                                    else x_hi[:, cch * 128 - XLO : (cch + 1) * 128 - XLO]
                                )
                                nc.tensor.transpose(
                                    tr[:, cch * 128 : (cch + 1) * 128], src, ident[:]
                                )
                            nc.vector.tensor_copy(
                                xT[:, :, tc4 * 128 : tc4 * 128 + 128],
                                tr[:].rearrange("p (c f) -> p c f", c=CCH),
                            )
                            first = ts == 0 and tc4 == 0
                            last = ts == NTS - 1 and tc4 == 3
                            seen_banks = set()
                            for row, plo, slo, w in G_A_SPLITS:
                                lhsT = x_lo[:, row * 128 : (row + 1) * 128]
                                rhs = (
                                    x_lo[:, slo : slo + w]
                                    if slo < XLO
                                    else x_hi[:, slo - XLO : slo - XLO + w]
                                )
                                bank = plo // 512
                                st = first and bank not in seen_banks
                                seen_banks.add(bank)
                                nc.tensor.matmul(
                                    g_acc[:, plo : plo + w],
                                    lhsT,
                                    rhs,
                                    start=st,
                                    stop=last,
                                    skip_group_check=True,
                                )
                        # ---- q^T chunks with fused gelu ----
                        gq = gqpool.tile([128, CCH, 512], BF16, tag="gq")
                        gq_ts.append(gq)
                        for jch in range(CCH):
                            pq = ps_pq.tile([128, 512], F32, tag="pq")
                            for cch in range(CCH):
                                nc.tensor.matmul(
                                    pq[:],
                                    w_qkv[:, cch, jch * 128 : (jch + 1) * 128],
                                    xT[:, cch, :],
                                    start=(cch == 0),
                                    stop=(cch == CCH - 1),
                                )
                            nc.scalar.activation(gq[:, jch, :], pq[:], GELU)

                    # ===== pass 1b: G rows 3-5 from retained hi tiles =======
                    gB1 = ps_pq.tile([128, 512], F32, tag="pq", name="gB1")
                    gB2 = ps_pq.tile([128, 256], F32, tag="pq", name="gB2")
                    g_b = (gB1, gB2)
                    mirrors = list(MIRRORS_EARLY)[::-1]
                    for i3 in range(3):  # G rows 0-2 evac (DVE/ACT mix)
                        w = C - i3 * 128
                        plo = (0, 768, 1408)[i3]
                        if i3 == 1:
                            nc.scalar.activation(
                                G_sb[:, i3, i3 * 128 : C], g_acc[:, plo : plo + w], COPY
                            )
                        else:
                            nc.vector.tensor_copy(
                                G_sb[:, i3, i3 * 128 : C], g_acc[:, plo : plo + w]
                            )
                    for tci in range(32):
                        x_hi = x_hi_keep[tci]
                        first = tci == 0
                        last = tci == 31
                        for row, tidx, plo, slo, w in G_B_SPLITS:
                            nc.tensor.matmul(
                                g_b[tidx][:, plo : plo + w],
                                x_hi[:, row * 128 - XLO : (row + 1) * 128 - XLO],
                                x_hi[:, slo : slo + w],
                                start=(first and plo == 0),
                                stop=last,
                                skip_group_check=True,
                            )
                        if tci >= 3 and tci % 3 >= 1 and mirrors:
                            i, j = mirrors.pop()
                            pt = ps_tr.tile([128, 128], BF16, tag="tr", name="pt")
                            nc.tensor.transpose(
                                pt[:], G_sb[:, j, i * 128 : i * 128 + 128], ident[:]
                            )
                            nc.vector.tensor_copy(
                                G_sb[:, i, j * 128 : j * 128 + 128], pt[:]
                            )
                    while mirrors:
                        i, j = mirrors.pop()
                        pt = ps_tr.tile([128, 128], BF16, tag="tr", name="pt")
                        nc.tensor.transpose(
                            pt[:], G_sb[:, j, i * 128 : i * 128 + 128], ident[:]
                        )
                        nc.vector.tensor_copy(
                            G_sb[:, i, j * 128 : j * 128 + 128], pt[:]
                        )
                    # rows 3-5 evac + remaining mirrors
                    nc.vector.tensor_copy(G_sb[:, 3, 384:768], gB1[:, 0:384])
                    nc.scalar.activation(G_sb[:, 4, 512:768], gB2[:], COPY)
                    nc.vector.tensor_copy(G_sb[:, 5, 640:768], gB1[:, 384:512])
                    for n, (i, j) in enumerate(MIRRORS_LATE):
                        pt = ps_tr.tile([128, 128], BF16, tag="tr", name="ptL")
                        nc.tensor.transpose(
                            pt[:], G_sb[:, j, i * 128 : i * 128 + 128], ident[:]
                        )
                        if n % 2 == 0:
                            nc.vector.tensor_copy(
                                G_sb[:, i, j * 128 : j * 128 + 128], pt[:]
                            )
                        else:
                            nc.scalar.activation(
                                G_sb[:, i, j * 128 : j * 128 + 128], pt[:], COPY
                            )

                if DEBUG_DUMPS and b == 0:
                    nc.sync.dma_start(dbg["G"][:], G_sb[:])
                    nc.sync.dma_start(dbg["gq"][:], gq_ts[0][:])

                # ===== chain: A = G @ W_k; kv^T = W_v^T A; W' ===============
                nc.gpsimd.memset(gkvT[:], 0.0)
                A_sb = apool.tile([128, CCH, C], BF16, tag="A")
                W_sb = apool.tile([128, NPAIR, C], BF16, tag="Wp")
                with tc.tile_pool(name="ps_post", bufs=2, space=PSUM) as ps_post:
                    for cp in range(CCH):
                        pA = ps_post.tile([128, C], F32, tag="post")
                        for lo, hi in ((0, 512), (512, 768)):
                            for cch in range(CCH):
                                nc.tensor.matmul(
                                    pA[:, lo:hi],
                                    G_sb[:, cch, cp * 128 : (cp + 1) * 128],
                                    w_qkv[:, cch, C + lo : C + hi],
                                    start=(cch == 0),
                                    stop=(cch == CCH - 1),
                                    skip_group_check=True,
                                )
                        nc.vector.tensor_copy(A_sb[:, cp, :], pA[:])

                    kv_acc = ps_post.tile([128, NPAIR * 128], F32, tag="post")
                    for pr in range(NPAIR):
                        psl = slice(pr * 128, pr * 128 + 128)
                        for cch in range(CCH):
                            # start clears the whole bank: first MM per bank only
                            nc.tensor.matmul(
                                kv_acc[:, psl],
                                w_qkv[:, cch, 2 * C + pr * 128 : 2 * C + (pr + 1) * 128],
                                A_sb[:, cch, pr * 128 : (pr + 1) * 128],
                                start=(cch == 0 and pr in (0, 4)),
                                stop=(cch == CCH - 1),
                                skip_group_check=True,
                            )
                    # gelu(kv^T * scale) into block-diagonal pair tiles (two
                    # batched ACTs: even-head halves, odd-head halves), then
                    # W'_pair = gkvT^T @ w_proj rows.
                    kv_v = kv_acc[:].rearrange("p (n f) -> p n f", n=NPAIR)
                    nc.scalar.activation(
                        gkvT[0:64, :, 0:64], kv_v[0:64, :, 0:64], GELU, scale=SCALE
                    )
                    nc.scalar.activation(
                        gkvT[64:128, :, 64:128], kv_v[64:128, :, 64:128], GELU,
                        scale=SCALE,
                    )
                    for pr in range(NPAIR):
                        pW = ps_post.tile([128, C], F32, tag="post", name="pW")
                        for lo, hi in ((0, 512), (512, 768)):
                            # each split is the first MM into its own bank
                            nc.tensor.matmul(
                                pW[:, lo:hi],
                                gkvT[:, pr, :],
                                w_proj[:, pr, lo:hi],
                                start=True,
                                stop=True,
                                skip_group_check=True,
                            )
                        if pr % 2 == 0:
                            nc.vector.tensor_copy(W_sb[:, pr, :], pW[:])
                        else:
                            nc.scalar.activation(W_sb[:, pr, :], pW[:], COPY)

                    if DEBUG_DUMPS and b == 0:
                        nc.sync.dma_start(dbg["A"][:], A_sb[:])
                        nc.sync.dma_start(dbg["W"][:], W_sb[:])

                    # ================= pass 2: y = gq @ W' + b ==============
                    for ts in range(NTS):
                        gq = gq_ts[ts]
                        for tc4 in range(4):
                            tsl = slice(tc4 * 128, tc4 * 128 + 128)
                            py = ps_post.tile([128, C], F32, tag="post", name="py")
                            for pr in range(NPAIR):
                                lastp = pr == NPAIR - 1
                                nc.tensor.matmul(
                                    py[:, 0:512],
                                    gq[:, pr, tsl],
                                    W_sb[:, pr, 0:512],
                                    start=(pr == 0),
                                    stop=lastp,
                                    skip_group_check=True,
                                )
                                nc.tensor.matmul(
                                    py[:, 512:768],
                                    gq[:, pr, tsl],
                                    W_sb[:, pr, 512:768],
                                    start=(pr == 0),
                                    stop=lastp,
                                    skip_group_check=True,
                                )
                            y_sb = ypool.tile([128, C], F32)
                            nc.vector.tensor_add(
                                y_sb[:, 0:512], py[:, 0:512], b_bc[:, 0:512]
                            )
                            nc.scalar.activation(y_sb[:, 512:768], py[:, 512:768], COPY)
                            nc.gpsimd.tensor_add(
                                y_sb[:, 512:768], y_sb[:, 512:768], b_bc[:, 512:768]
                            )
                            t0 = ts * 512 + tc4 * 128
                            nc.sync.dma_start(y_d[b, t0 : t0 + 128, :], y_sb[:])

    nc.compile()
    return nc


_cached_nc = None


def kernel(x, w_qkv, w_proj, b_proj):
    global _cached_nc
    if _cached_nc is None:
        _cached_nc = _build_program()
    nc = _cached_nc

    x = np.ascontiguousarray(x, dtype=np.float32)
    in_maps = [
        {
            "x": x[i * BPC : (i + 1) * BPC],
            "w_qkv": np.asarray(w_qkv, dtype=np.float32),
            "w_proj": np.asarray(w_proj, dtype=np.float32),
            "b_proj": np.asarray(b_proj, dtype=np.float32),
        }
        for i in range(NCORES)
    ]
    last_err = None
    for _attempt in range(3):
        try:
            res = run_bass_kernel_spmd(nc, in_maps, core_ids=list(range(NCORES)))
            out = np.concatenate(
                [res.results[i]["y"] for i in range(NCORES)], axis=0
            )
            return out.astype(np.float32)
        except Exception as e:  # transient NRT device errors recover on retry
            last_err = e
    raise last_err
